# revision 11
# baseline (speedup 1.0000x reference)
"""Cross-attention kernel for Trainium2, 8 NeuronCores, data-parallel over batch.

Problem (per batch element b, one per core):
    q  = x_b @ Wq.T + bq                      [T=1024, C=1024]
    kv = enc_b @ Wkv.T + bkv                  [I=576, 2C]
    per head h (H=16, D=64):
        att = softmax((q_h @ k_h.T) / sqrt(D))
        y_h = att @ v_h
    out = y @ Wo.T + bo                       [T, C]

Design notes:
  - One batch element per core (B=8 == n_cores), no collectives.
  - Weights are pre-transposed on host to [in, out] layout so the
    contraction dim (c) lands on SBUF partitions for matmuls.
  - x / enc are transposed on-device via the PE (out = in.T @ I).
  - Matmuls run as float32r (TF32-like, 1 cyc/row at N>=256) via AP bitcast.
  - Attention is computed in S^T = K_h @ Q_h^T orientation ([i, t]); exp is
    applied without max-subtraction (scores are O(1), exp <= ~e^6).  The
    softmax denominator Z_t falls out of the AV matmul by augmenting V with
    a ones column (lhsT M=65); normalization multiplies y^T by a rank-1
    PE-broadcast of 1/Z.
  - Biases: bq/bk are per-partition adds; bv/bo are rank-1 (K=1) matmul
    accumulates of ones^T (x) bias_row.
"""

import numpy as np

T = 1024
C = 1024
I = 576
H = 16
D = 64
NCC = C // 128          # 8 contraction chunks
NIC = (I + 127) // 128  # 5 i chunks (128,128,128,128,64)
I_CH = [128, 128, 128, 128, 64]
VW = 68                 # per-head column block in V tile: 64 v cols + ones col + pad
SCALE = 1.0 / np.sqrt(D)

_CACHE = {}


def _build_nc():
    import concourse.bass as bass
    import concourse.bacc as bacc
    import concourse.mybir as mybir
    import concourse.tile as tile
    from contextlib import ExitStack

    f32 = mybir.dt.float32
    f32r = mybir.dt.float32r

    nc = bacc.Bacc()

    x_d = nc.dram_tensor("x", [T, C], f32r, kind="ExternalInput")
    enc_d = nc.dram_tensor("enc", [I, C], f32r, kind="ExternalInput")
    wqT_d = nc.dram_tensor("wqT", [C, C], f32r, kind="ExternalInput")
    wkT_d = nc.dram_tensor("wkT", [C, C], f32r, kind="ExternalInput")
    wvT_d = nc.dram_tensor("wvT", [C, C], f32r, kind="ExternalInput")
    woT_d = nc.dram_tensor("woT", [C, C], f32r, kind="ExternalInput")
    bq_d = nc.dram_tensor("bq", [C], f32, kind="ExternalInput")
    bk_d = nc.dram_tensor("bk", [C], f32, kind="ExternalInput")
    bv_d = nc.dram_tensor("bv", [C], f32r, kind="ExternalInput")
    bo_d = nc.dram_tensor("bo", [C], f32r, kind="ExternalInput")
    out_d = nc.dram_tensor("out", [T, C], f32, kind="ExternalOutput")

    with ExitStack() as ctx:
        tc = ctx.enter_context(tile.TileContext(nc))

        # long-lived pools
        resid = ctx.enter_context(tc.tile_pool(name="resid", bufs=1))
        misc = ctx.enter_context(tc.tile_pool(name="misc", bufs=1))
        pa = ctx.enter_context(tc.tile_pool(name="pa", bufs=6, space="PSUM"))
        exps = ctx.enter_context(tc.tile_pool(name="exps", bufs=10))

        # constants (DMA'd from NEFF-embedded data; engines can't memset f32r)
        ident_d = nc.inline_tensor(np.eye(128, dtype=np.float32), name="ident_d")
        ones_d = nc.inline_tensor(np.ones((128, 128), dtype=np.float32), name="ones_d")
        ident = misc.tile([128, 128], f32r)
        nc.sync.dma_start(out=ident, in_=ident_d[:, :].bitcast(f32r))
        ones_t = misc.tile([128, 128], f32r)
        nc.sync.dma_start(out=ones_t, in_=ones_d[:, :].bitcast(f32r))
        bq_t = misc.tile([128, NCC], f32)
        nc.sync.dma_start(out=bq_t, in_=bq_d[:].rearrange("(oc p) -> p oc", p=128))
        bk_t = misc.tile([128, NCC], f32)
        nc.sync.dma_start(out=bk_t, in_=bk_d[:].rearrange("(oc p) -> p oc", p=128))
        bv_row = misc.tile([1, C], f32r)
        nc.sync.dma_start(out=bv_row, in_=bv_d[:].unsqueeze(0))
        bo_row = misc.tile([1, C], f32r)
        nc.sync.dma_start(out=bo_row, in_=bo_d[:].unsqueeze(0))

        # resident tensors
        QT = [resid.tile([128, T], f32r, tag=f"QT{i}", name=f"QT{i}") for i in range(NCC)]
        KT = [resid.tile([128, I], f32r, tag=f"KT{i}", name=f"KT{i}") for i in range(NCC)]
        V3 = [resid.tile([128, H, VW], f32r, tag=f"V{i}", name=f"V{i}") for i in range(NIC)]
        YT = [resid.tile([128, T], f32r, tag=f"YT{i}", name=f"YT{i}") for i in range(NCC)]

        with tc.tile_pool(name="ph1", bufs=1) as ph1, \
             tc.tile_pool(name="xin", bufs=3) as xin, \
             tc.tile_pool(name="wsm", bufs=4) as wsm, \
             tc.tile_pool(name="wv8", bufs=1) as wv8, \
             tc.tile_pool(name="pt", bufs=2, space="PSUM") as pt:

            # ---- enc^T (resident through V proj) ----
            encT = [ph1.tile([128, I], f32r, tag=f"encT{i}", name=f"encT{i}") for i in range(NCC)]
            for ii in range(NIC):
                pi = I_CH[ii]
                e_nat = xin.tile([128, C], f32r, tag="xin")
                nc.sync.dma_start(out=e_nat[:pi], in_=enc_d[ii * 128 : ii * 128 + pi])
                for cc in range(NCC):
                    ps = pt.tile([128, 128], f32r, tag="pt")
                    nc.tensor.transpose(
                        ps[:128, :pi],
                        e_nat[:pi, cc * 128 : (cc + 1) * 128],
                        ident[:pi, :pi],
                    )
                    nc.vector.tensor_copy(
                        encT[cc][:, ii * 128 : ii * 128 + pi], ps[:128, :pi]
                    )

            # ---- x^T in t-halves + Q^T projection ----
            for tch in range(2):
                xTh = [ph1.tile([128, 512], f32r, tag=f"xTh{i}", name=f"xTh{i}") for i in range(NCC)]
                for ts in range(4):
                    tt = tch * 4 + ts
                    x_nat = xin.tile([128, C], f32r, tag="xin")
                    nc.sync.dma_start(out=x_nat, in_=x_d[tt * 128 : (tt + 1) * 128])
                    for cc in range(NCC):
                        ps = pt.tile([128, 128], f32r, tag="pt")
                        nc.tensor.transpose(
                            ps, x_nat[:, cc * 128 : (cc + 1) * 128], ident
                        )
                        nc.vector.tensor_copy(
                            xTh[cc][:, ts * 128 : (ts + 1) * 128], ps
                        )
                # Q^T[o, t-half] = (WqT).T @ x^T ; accumulate over c chunks
                for oc in range(NCC):
                    pq = pa.tile([128, 512], f32, tag="pa")
                    for cc in range(NCC):
                        wch = wsm.tile([128, 128], f32r, tag="wsm")
                        nc.sync.dma_start(
                            out=wch,
                            in_=wqT_d[
                                cc * 128 : (cc + 1) * 128, oc * 128 : (oc + 1) * 128
                            ],
                        )
                        nc.tensor.matmul(
                            pq,
                            wch,
                            xTh[cc],
                            start=(cc == 0),
                            stop=(cc == NCC - 1),
                        )
                    nc.vector.tensor_scalar_add(
                        QT[oc][:, tch * 512 : (tch + 1) * 512],
                        pq,
                        bq_t[:, oc : oc + 1],
                    )

            # ---- K^T projection (i in halves of 288) ----
            for oc in range(NCC):
                pk = [pa.tile([128, 288], f32, tag="pa", name=f"pk{_}") for _ in range(2)]
                for cc in range(NCC):
                    wch = wsm.tile([128, 128], f32r, tag="wsm")
                    nc.sync.dma_start(
                        out=wch,
                        in_=wkT_d[
                            cc * 128 : (cc + 1) * 128, oc * 128 : (oc + 1) * 128
                        ],
                    )
                    for ih in range(2):
                        nc.tensor.matmul(
                            pk[ih],
                            wch,
                            encT[cc][:, ih * 288 : (ih + 1) * 288],
                            start=(cc == 0),
                            stop=(cc == NCC - 1),
                        )
                for ih in range(2):
                    nc.vector.tensor_scalar_add(
                        KT[oc][:, ih * 288 : (ih + 1) * 288],
                        pk[ih],
                        bk_t[:, oc : oc + 1],
                    )

            # ---- V projection into [128, H, VW] layout with ones columns ----
            for ii in range(NIC):
                # ones column (head-block col 64) for the fused Z row in AV
                nc.sync.dma_start(
                    out=V3[ii][:, :, 64:65],
                    in_=ones_d[:, 0:H].bitcast(f32r).unsqueeze(2),
                )
            for och in range(2):
                wvt = [wv8.tile([128, 512], f32r, tag=f"wv{i}", name=f"wv{i}") for i in range(NCC)]
                for cc in range(NCC):
                    nc.sync.dma_start(
                        out=wvt[cc],
                        in_=wvT_d[cc * 128 : (cc + 1) * 128, och * 512 : (och + 1) * 512],
                    )
                for ii in range(NIC):
                    pi = I_CH[ii]
                    pv = pa.tile([128, 512], f32, tag="pa")
                    for cc in range(NCC):
                        nc.tensor.matmul(
                            pv[:pi],
                            encT[cc][:, ii * 128 : ii * 128 + pi],
                            wvt[cc],
                            start=(cc == 0),
                            stop=False,
                        )
                    # bv: rank-1 ones^T (x) bv_row accumulate
                    nc.tensor.matmul(
                        pv[:pi],
                        ones_t[0:1, :pi],
                        bv_row[0:1, och * 512 : (och + 1) * 512],
                        start=False,
                        stop=True,
                    )
                    dst = V3[ii][:pi, och * 8 : och * 8 + 8, 0:64]
                    nc.vector.tensor_copy(
                        dst, pv[:pi].rearrange("p (h d) -> p h d", d=64)
                    )

        # ---- attention ----
        with tc.tile_pool(name="attn", bufs=3) as attn:
            for h in range(H):
                oc = h // 2
                hb = (h % 2) * 64
                for tch in range(2):
                    tsl = slice(tch * 512, (tch + 1) * 512)
                    # S^T chunks -> exp -> sbuf
                    es = []
                    for ii in range(NIC):
                        pi = I_CH[ii]
                        ps = pa.tile([128, 512], f32, tag="pa")
                        nc.tensor.matmul(
                            ps[:pi],
                            KT[oc][hb : hb + 64, ii * 128 : ii * 128 + pi],
                            QT[oc][hb : hb + 64, tsl],
                            start=True,
                            stop=True,
                        )
                        e = exps.tile([128, 512], f32r, tag="exps")
                        nc.scalar.activation(
                            e[:pi],
                            ps[:pi],
                            mybir.ActivationFunctionType.Exp,
                            scale=float(SCALE),
                        )
                        es.append(e)
                    # y^T (64 rows) and Z (row 64) via V augmented with ones col
                    py = pa.tile([128, 512], f32, tag="pa")
                    for ii in range(NIC):
                        pi = I_CH[ii]
                        nc.tensor.matmul(
                            py[:65],
                            V3[ii][:pi, h, 0:65],
                            es[ii][:pi],
                            start=(ii == 0),
                            stop=(ii == NIC - 1),
                        )
                    # r = 1/Z on partition 64; rank-1 broadcast to [64, 512]
                    rz = attn.tile([128, 512], f32r, tag="rz")
                    with nc.allow_low_precision(reason="1/Z in f32r is fine"):
                        nc.vector.reciprocal(rz[64:65], py[64:65])
                    pb = pa.tile([128, 512], f32, tag="pa")
                    nc.tensor.matmul(
                        pb[:64],
                        ones_t[64:65, 0:64],
                        rz[64:65],
                        start=True,
                        stop=True,
                    )
                    zb = attn.tile([64, 512], f32, tag="zb")
                    nc.vector.tensor_copy(zb, pb[:64])
                    nc.vector.tensor_mul(YT[oc][hb : hb + 64, tsl], py[:64], zb)

        # ---- output projection ----
        with tc.tile_pool(name="wo16", bufs=1) as wo16, \
             tc.tile_pool(name="osb", bufs=3) as osb:
            wot = {}
            for cc in range(NCC):
                for och in range(2):
                    w = wo16.tile([128, 512], f32r, tag=f"wo{cc}_{och}", name=f"wo{cc}_{och}")
                    nc.sync.dma_start(
                        out=w,
                        in_=woT_d[
                            cc * 128 : (cc + 1) * 128, och * 512 : (och + 1) * 512
                        ],
                    )
                    wot[(cc, och)] = w
            for tt in range(8):
                ot = osb.tile([128, C], f32, tag="osb")
                for och in range(2):
                    po = pa.tile([128, 512], f32, tag="pa")
                    for cc in range(NCC):
                        nc.tensor.matmul(
                            po,
                            YT[cc][:, tt * 128 : (tt + 1) * 128],
                            wot[(cc, och)],
                            start=(cc == 0),
                            stop=False,
                        )
                    nc.tensor.matmul(
                        po,
                        ones_t[0:1, 0:128],
                        bo_row[0:1, och * 512 : (och + 1) * 512],
                        start=False,
                        stop=True,
                    )
                    nc.vector.tensor_copy(ot[:, och * 512 : (och + 1) * 512], po)
                nc.sync.dma_start(out=out_d[tt * 128 : (tt + 1) * 128], in_=ot)

    nc.compile()
    return nc


def _get_nc():
    if "nc" not in _CACHE:
        _CACHE["nc"] = _build_nc()
    return _CACHE["nc"]


def _prep_in_maps(x, encoder_output, Wq, bq, Wkv, bkv, Wo, bo):
    f = np.float32
    x = np.asarray(x, f)
    enc = np.asarray(encoder_output, f)
    wqT = np.ascontiguousarray(np.asarray(Wq, f).T)
    wkv = np.asarray(Wkv, f)
    wkT = np.ascontiguousarray(wkv[:C].T)
    wvT = np.ascontiguousarray(wkv[C:].T)
    woT = np.ascontiguousarray(np.asarray(Wo, f).T)
    bq = np.asarray(bq, f)
    bkv = np.asarray(bkv, f)
    bo = np.asarray(bo, f)
    shared = {
        "wqT": wqT, "wkT": wkT, "wvT": wvT, "woT": woT,
        "bq": bq, "bk": np.ascontiguousarray(bkv[:C]),
        "bv": np.ascontiguousarray(bkv[C:]), "bo": bo,
    }
    return [
        dict(shared, x=np.ascontiguousarray(x[b]), enc=np.ascontiguousarray(enc[b]))
        for b in range(x.shape[0])
    ]


def kernel(x, encoder_output, Wq, bq, Wkv, bkv, Wo, bo):
    from concourse.bass_utils import run_bass_kernel_spmd

    nc = _get_nc()
    in_maps = _prep_in_maps(x, encoder_output, Wq, bq, Wkv, bkv, Wo, bo)
    res = run_bass_kernel_spmd(nc, in_maps, list(range(len(in_maps)))).results
    return np.stack([res[b]["out"] for b in range(len(res))]).astype(np.float32)
